# revision 15
# baseline (speedup 1.0000x reference)
"""Trainium2 Bass kernel for nn_BaGuaModel (4-layer BaGua transformer encoder
with ZuoEr sequential memory and mean-pooled classifier head).

Sharding: data-parallel over batch B=8 across the 8 NeuronCores (one sequence
per core). Small params are replicated; each core runs the full forward for
its sequence and returns the [D]-sized mean-pooled scan output; the final
(tiny) classifier LayerNorm + linear runs on host over the gathered [8, D].

On-device layout: activations are kept transposed, x^T : [D(=partitions,
4 chunks of 128), S(=free, 2048)], so every matmul contracts over the
partition dimension with no transposes between layers.

Key optimizations over the straightforward version:
  - embedding gather via ONE gpsimd dma_gather(transpose=True) on a bf16
    copy of the table (replaces 16 indirect DMAs + 64 PE transposes + 64
    PSUM-read adds)
  - cos(res_freq*pi) folded into tri_W; head-mixing folded into a single
    on-device-built W3 so the heads tensor is never materialized
  - LN2 (the FF pre-LN) eliminated: with ln1_g==1/ln1_b==0 the LN1 output
    already has zero mean/unit variance per column, so LN2 is the identity
    up to O(eps)=1e-5; LN1 writes the bf16 FF input directly
  - LN1 row stats via PE ones-matmuls; rsqrt via one Act Rsqrt; the
    per-column scale/shift broadcasts run on the (otherwise idle) Pool
    engine via partition_broadcast, keeping the DVE applies all-SBUF
    (2x DVE rate)
  - every sigmoid is computed as tanh (sig(z) = (1+tanh(z/2))/2, algebra
    folded into neighbors) and the tiny pol-chain gelu/softplus use
    Taylor forms, so the Act engine needs only 2 table switches per layer
  - FF and SuanLi emission interleaved per 512-column block so PE work of
    the next block overlaps Act/DVE work of the previous one
  - the ZuoEr recurrence is one DVE tensor_tensor_scan; its sigmoids are
    tanh-folded into the host-prescaled weights

Matmuls run in float32r (full-rate fp32 at free size >= 256); the FF block
runs in bf16.
"""
import os
import sys

sys.path.insert(0, "/opt/trn_rl_repo")

import numpy as np
from contextlib import ExitStack

import concourse.bass as bass
import concourse.tile as tile
from concourse import bacc, mybir
from concourse import bass_utils

F32 = mybir.dt.float32
F32R = mybir.dt.float32r
BF16 = mybir.dt.bfloat16
I16 = mybir.dt.int16
AF = mybir.ActivationFunctionType
ALU = mybir.AluOpType
AX = mybir.AxisListType

V, D, KH, L, PP, MEM, FF, S, B, C = 32000, 512, 64, 4, 32, 16, 2048, 2048, 8, 4
NCORES = 8
SB = 512              # s-block (psum free size)
NSB = S // SB         # 4
DC = D // 128         # 4 feature chunks
FFC = FF // 128       # 16 ff chunks
LN2_2 = 0.6931471805599453


def _build(fast_flags, debug_outs: bool, nlayers: int = L, do_scan: bool = True):
    ident_ln, taylor_ok = fast_flags
    nc = bacc.Bacc("TRN2", target_bir_lowering=False, debug=False,
                   num_devices=NCORES)

    def din(name, shape, dt=F32):
        return nc.dram_tensor(name, list(shape), dt, kind="ExternalInput")

    idx_d = din("idx", [128, S // 128], mybir.dt.int32)
    emb_d = din("emb", [V, D])
    posN_d = din("posN", [S, D])
    i128_d = din("i128", [128, 128])
    triWT_d = din("triWT", [L * D, D])    # [d, (hk)] per layer
    triWN_d = din("triWN", [L * D, D])    # [(hk), d] per layer
    outWT_d = din("outWT", [L * D, D])
    outb_d = din("outb", [L * D, 1])
    ln1g_d = din("ln1g", [L * D, 1])
    ln1negg_d = din("ln1negg", [L * D, 1])
    ln1b_d = din("ln1b", [L * D, 1])
    w1pT_d = din("w1pT", [L * D, FF], BF16)
    b1p_d = din("b1p", [L * 128, FFC])
    ffW2T_d = din("ffW2T", [L * FF, D], BF16)
    ffb2_d = din("ffb2", [L * D, 1])
    disWT_d = din("disWT", [D, D])
    chgWT_d = din("chgWT", [D, D])
    disbh_d = din("disbh", [D, 1])        # dis_b / 2 (tanh form)
    chgbh_d = din("chgbh", [D, 1])        # chg_b / 2
    zwr33_d = din("zwr33", [D, 65])
    zwrbh_d = din("zwrbh", [65, 1])       # halved biases (tanh form)
    zpWTh_d = din("zpWTh", [MEM, D])      # zp_W^T / 2
    zpb_d = din("zpb", [D, 1])
    zoWT_d = din("zoWT", [2 * D, D])
    zob_d = din("zob", [D, 1])
    e8_d = din("e8", [8, D])
    kron_d = din("kron", [D, D])
    mask01_d = din("mask01", [8, 8])
    i8_d = din("i8", [8, 8])
    w1f_d = din("w1full", [L * 8, 128])
    b1f_d = din("b1full", [L * 8, 128])
    w2f_d = din("w2full", [L * 8, 128])
    b2r_d = din("b2rep", [L * 8, 1])
    polWp_d = din("polWp", [L * D, PP])
    polb_d = din("polb", [L * PP, 8])
    b8_d = din("b8", [D, 8])
    onesv_d = din("onesv", [128, 128])
    onesb_d = din("onesb", [128, 1], BF16)

    pooled_d = nc.dram_tensor("pooled", [D, 1], F32, kind="ExternalOutput")
    dbg = {}
    if debug_outs:
        for nm in ["x0", "x2", "x3", "x4"]:
            dbg[nm] = nc.dram_tensor("dbg_" + nm, [D, S], F32,
                                     kind="ExternalOutput")
        dbg["x1"] = nc.dram_tensor("dbg_x1", [D, S], BF16,
                                   kind="ExternalOutput")
        dbg["ys"] = nc.dram_tensor("dbg_ys", [D, S], BF16,
                                   kind="ExternalOutput")
        dbg["scan"] = nc.dram_tensor("dbg_scan", [MEM, S], F32,
                                     kind="ExternalOutput")
        dbg["coef"] = nc.dram_tensor("dbg_coef", [8, 8], F32,
                                     kind="ExternalOutput")

    with tile.TileContext(nc) as tc, ExitStack() as ctx:
        # ---------- pools that live for the whole kernel ----------
        actp = ctx.enter_context(tc.tile_pool(name="act", bufs=1))
        cstp = ctx.enter_context(tc.tile_pool(name="cst", bufs=1))
        smp = ctx.enter_context(tc.tile_pool(name="small", bufs=1))
        psb = ctx.enter_context(tc.tile_pool(name="psb", bufs=4, space="PSUM"))

        # big activation tensors, 4 chunks of [128, S] each
        A = [actp.tile([128, S], F32R, tag=f"A{i}", name=f"A{i}") for i in range(DC)]
        Bt = [actp.tile([128, S], F32R, tag=f"B{i}", name=f"B{i}") for i in range(DC)]
        Ct = [actp.tile([128, S], BF16, tag=f"C{i}", name=f"C{i}") for i in range(DC)]

        # constants
        i128 = cstp.tile([128, 128], F32, tag="i128", name="i128")
        nc.sync.dma_start(i128[:], i128_d[:, :])
        e8r = cstp.tile([8, D], F32R, tag="e8", name="e8")
        nc.sync.dma_start(e8r[:], e8_d[:, :].bitcast(F32R))
        kron = [cstp.tile([128, D], F32R, tag=f"kron{i}", name=f"kron{i}") for i in range(DC)]
        for i in range(DC):
            nc.sync.dma_start(kron[i][:],
                              kron_d[i * 128:(i + 1) * 128, :].bitcast(F32R))
        mask01 = cstp.tile([8, 8], F32, tag="mask01", name="mask01")
        nc.sync.dma_start(mask01[:], mask01_d[:, :])
        i8 = cstp.tile([8, 8], F32, tag="i8", name="i8")
        nc.sync.dma_start(i8[:], i8_d[:, :])
        b8 = [cstp.tile([128, 8], F32, tag=f"b8{i}", name=f"b8{i}") for i in range(DC)]
        for i in range(DC):
            nc.sync.dma_start(b8[i][:],
                              b8_d[i * 128:(i + 1) * 128, :])
        ones128 = cstp.tile([128, 1], F32R, tag="ones128", name="ones128")
        nc.sync.dma_start(ones128[:], onesv_d[:, 0:1].bitcast(F32R))
        ones128b = cstp.tile([128, 1], BF16, tag="ones128b", name="ones128b")
        nc.sync.dma_start(ones128b[:], onesb_d[:, :])
        ones1x128 = cstp.tile([1, 128], F32R, tag="ones1x128", name="ones1x128")
        nc.sync.dma_start(ones1x128[:], onesv_d[0:1, :].bitcast(F32R))
        ones116 = cstp.tile([1, MEM], F32R, tag="ones116", name="ones116")
        nc.sync.dma_start(ones116[:], onesv_d[0:1, 0:MEM].bitcast(F32R))
        onecol = cstp.tile([128, 1], F32, tag="onecol", name="onecol")
        nc.vector.memset(onecol[:], 1.0)
        negcol = cstp.tile([128, 1], F32, tag="negcol", name="negcol")
        nc.vector.memset(negcol[:], -1.0)
        epsrow = cstp.tile([1, 1], F32, tag="epsrow", name="epsrow")
        nc.vector.memset(epsrow[:], 1e-5)

        # shared (layer-independent) weights, resident
        diswt = [cstp.tile([128, D], F32R, tag=f"dis{i}", name=f"dis{i}") for i in range(DC)]
        chgwt = [cstp.tile([128, D], F32R, tag=f"chg{i}", name=f"chg{i}") for i in range(DC)]
        for i in range(DC):
            nc.sync.dma_start(diswt[i][:],
                              disWT_d[i * 128:(i + 1) * 128, :].bitcast(F32R))
            nc.sync.dma_start(chgwt[i][:],
                              chgWT_d[i * 128:(i + 1) * 128, :].bitcast(F32R))
        disbh = [cstp.tile([128, 1], F32, tag=f"disb{i}", name=f"disb{i}") for i in range(DC)]
        chgbh = [cstp.tile([128, 1], F32, tag=f"chgb{i}", name=f"chgb{i}") for i in range(DC)]
        for i in range(DC):
            nc.sync.dma_start(disbh[i][:], disbh_d[i * 128:(i + 1) * 128, :])
            nc.sync.dma_start(chgbh[i][:], chgbh_d[i * 128:(i + 1) * 128, :])

        xbparts = [smp.tile([128, S // 128], F32, tag=f"xbp{i}",
                            name=f"xbp{i}") for i in range(DC)]

        # ---------- embedding gather + pos add + transpose ----------
        with tc.tile_pool(name="embp", bufs=3) as embp, \
             tc.tile_pool(name="posp", bufs=3) as posp, \
             tc.tile_pool(name="tpsp", bufs=4, space="PSUM") as tpsp:
            idxt = embp.tile([128, S // 128], mybir.dt.int32, tag="idx",
                             name="idx")
            nc.sync.dma_start(idxt[:], idx_d[:, :])
            for g in range(S // 128):
                gt = embp.tile([128, D], F32, tag="embg", name="embg")
                nc.gpsimd.indirect_dma_start(
                    out=gt[:], out_offset=None, in_=emb_d[:, :],
                    in_offset=bass.IndirectOffsetOnAxis(ap=idxt[:, g:g + 1],
                                                        axis=0))
                pt = posp.tile([128, D], F32, tag="pos", name="pos")
                nc.sync.dma_start(pt[:], posN_d[g * 128:(g + 1) * 128, :])
                gp = posp.tile([128, D], F32, tag="gp", name="gp")
                nc.vector.tensor_tensor(out=gp[:], in0=gt[:], in1=pt[:],
                                        op=ALU.add)
                for dc in range(DC):
                    tp = tpsp.tile([128, 128], F32, tag="tps", name="tps")
                    nc.tensor.transpose(tp[:], gp[:, dc * 128:(dc + 1) * 128],
                                        i128[:])
                    with nc.allow_low_precision(reason="f32r storage"):
                        nc.scalar.activation(
                            A[dc][:, g * 128:(g + 1) * 128], tp[:],
                            AF.Identity,
                            accum_out=xbparts[dc][:, g:g + 1])
        if debug_outs:
            for dc in range(DC):
                nc.sync.dma_start(dbg["x0"][dc * 128:(dc + 1) * 128, :],
                                  A[dc][:].bitcast(F32))

        # ---------- layers ----------
        with tc.tile_pool(name="wlay", bufs=1) as wlay, \
             tc.tile_pool(name="wstr", bufs=6) as wstr, \
             tc.tile_pool(name="ev", bufs=2) as evp, \
             tc.tile_pool(name="ge", bufs=4) as gep, \
             tc.tile_pool(name="cgp", bufs=4) as cgpp, \
             tc.tile_pool(name="bcp", bufs=2) as bcp, \
             tc.tile_pool(name="rows", bufs=6) as rowp, \
             tc.tile_pool(name="psff", bufs=1, space="PSUM") as psff:

            # ---- fast LayerNorm (g=1, b=0): v (f32r) -> outC (bf16) ----
            def emit_ln_fast(v, outC, sblk):
                sl = slice(sblk * SB, (sblk + 1) * SB)
                st = psb.tile([33, SB], F32, tag="ps", name="lnst")
                for dc in range(DC):
                    nc.tensor.matmul(st[0:1, :], ones128[:], v[dc][:, sl],
                                     start=(dc == 0), stop=(dc == DC - 1))
                for dc in range(DC):
                    sqt = evp.tile([128, SB], BF16, tag="lnsq", name="lnsq")
                    nc.scalar.activation(sqt[:], v[dc][:, sl], AF.Square)
                    nc.tensor.matmul(st[32:33, :], ones128b[:], sqt[:],
                                     start=(dc == 0), stop=(dc == DC - 1))
                msq = rowp.tile([1, SB], F32, tag="lnrow", name="lnr1")
                nc.scalar.activation(msq[:], st[0:1, :], AF.Square,
                                     scale=1.0 / D)
                var = rowp.tile([1, SB], F32, tag="lnrow", name="lnr2")
                nc.vector.scalar_tensor_tensor(
                    out=var[:], in0=st[32:33, :], scalar=1.0 / D,
                    in1=msq[:], op0=ALU.mult, op1=ALU.subtract)
                sd = rowp.tile([1, SB], F32, tag="lnrow", name="lnr3")
                nc.scalar.activation(sd[:], var[:], AF.Sqrt, bias=epsrow[:])
                rsf = rowp.tile([1, SB], F32, tag="lnrow", name="lnr5")
                scr = rowp.tile([1, SB], F32, tag="lnrow", name="lnr6")
                nc.vector.reciprocal_approx_accurate(rsf[:], sd[:], scr[:])
                rs = rowp.tile([1, SB], F32R, tag="lnrow", name="lnr7")
                u = rowp.tile([1, SB], F32R, tag="lnrow", name="lnr4")
                with nc.allow_low_precision(reason="f32r for matmul"):
                    nc.vector.tensor_copy(rs[:], rsf[:])
                    nc.vector.scalar_tensor_tensor(
                        out=u[:], in0=st[0:1, :], scalar=1.0 / D,
                        in1=rsf[:], op0=ALU.mult, op1=ALU.mult)
                rsp = psb.tile([128, SB], F32, tag="ps", name="lnbc1p")
                nc.tensor.matmul(rsp[:], ones1x128[:],
                                 rs[:], start=True, stop=True)
                ubp = psb.tile([128, SB], F32, tag="ps", name="lnbc2p")
                nc.tensor.matmul(ubp[:], ones1x128[:],
                                 u[:], start=True, stop=True)
                rsb = bcp.tile([128, SB], F32, tag="bc1", name="lnbc1")
                nc.scalar.activation(rsb[:], rsp[:], AF.Identity)
                ub = bcp.tile([128, SB], F32, tag="bc2", name="lnbc2")
                nc.scalar.activation(ub[:], ubp[:], AF.Identity)
                for dc in range(DC):
                    tmp = evp.tile([128, SB], F32, tag="lntmp", name="lntmp")
                    nc.vector.tensor_tensor(out=tmp[:], in0=v[dc][:, sl],
                                            in1=rsb[:], op=ALU.mult)
                    with nc.allow_low_precision(reason="bf16 storage"):
                        nc.vector.tensor_tensor(out=outC[dc][:, sl],
                                                in0=tmp[:], in1=ub[:],
                                                op=ALU.subtract)

            # ---- general LayerNorm fallback (affine) ----
            def emit_ln(v, out, g_ap, negg_ap, b_ap):
                for sblk in range(NSB):
                    sl = slice(sblk * SB, (sblk + 1) * SB)
                    st = psb.tile([33, SB], F32, tag="ps", name="lnstg")
                    for dc in range(DC):
                        nc.tensor.matmul(st[0:1, :], ones128[:], v[dc][:, sl],
                                         start=(dc == 0), stop=(dc == DC - 1))
                    for dc in range(DC):
                        sqt = evp.tile([128, SB], BF16, tag="lnsq", name="lnsq")
                        nc.scalar.activation(sqt[:], v[dc][:, sl], AF.Square)
                        nc.tensor.matmul(st[32:33, :], ones128b[:], sqt[:],
                                         start=(dc == 0), stop=(dc == DC - 1))
                    msq = rowp.tile([1, SB], F32, tag="lnrow", name="lnr1")
                    nc.scalar.activation(msq[:], st[0:1, :], AF.Square,
                                         scale=1.0 / D)
                    var = rowp.tile([1, SB], F32, tag="lnrow", name="lnr2")
                    nc.vector.scalar_tensor_tensor(
                        out=var[:], in0=st[32:33, :], scalar=1.0 / D,
                        in1=msq[:], op0=ALU.mult, op1=ALU.subtract)
                    sd = rowp.tile([1, SB], F32, tag="lnrow", name="lnr3")
                    nc.scalar.activation(sd[:], var[:], AF.Sqrt,
                                         bias=epsrow[:])
                    rsf = rowp.tile([1, SB], F32, tag="lnrow", name="lnr5")
                    scr = rowp.tile([1, SB], F32, tag="lnrow", name="lnr6")
                    nc.vector.reciprocal_approx_accurate(rsf[:], sd[:], scr[:])
                    rs = rowp.tile([1, SB], F32R, tag="lnrow", name="lnr7")
                    u = rowp.tile([1, SB], F32R, tag="lnrow", name="lnr4")
                    with nc.allow_low_precision(reason="f32r for matmul"):
                        nc.vector.tensor_copy(rs[:], rsf[:])
                        nc.vector.scalar_tensor_tensor(
                            out=u[:], in0=st[0:1, :], scalar=1.0 / D,
                            in1=rsf[:], op0=ALU.mult, op1=ALU.mult)
                    rsp = psb.tile([128, SB], F32, tag="ps", name="lnb1p")
                    nc.tensor.matmul(rsp[:], ones1x128[:],
                                     rs[:], start=True, stop=True)
                    ubp = psb.tile([128, SB], F32, tag="ps", name="lnb2p")
                    nc.tensor.matmul(ubp[:], ones1x128[:],
                                     u[:], start=True, stop=True)
                    rsb = bcp.tile([128, SB], F32, tag="bc1", name="lnbc1")
                    nc.scalar.activation(rsb[:], rsp[:], AF.Identity)
                    ub = bcp.tile([128, SB], F32, tag="bc2", name="lnbc2")
                    nc.scalar.activation(ub[:], ubp[:], AF.Identity)
                    for dc in range(DC):
                        tmp = evp.tile([128, SB], F32, tag="lntmp",
                                       name="lntmp")
                        nc.vector.scalar_tensor_tensor(
                            out=tmp[:], in0=v[dc][:, sl], scalar=g_ap[dc],
                            in1=rsb[:], op0=ALU.mult, op1=ALU.mult)
                        with nc.allow_low_precision(reason="storage"):
                            nc.vector.scalar_tensor_tensor(
                                out=out[dc][:, sl], in0=ub[:],
                                scalar=negg_ap[dc], in1=tmp[:],
                                op0=ALU.mult, op1=ALU.add)
                            if b_ap is not None:
                                nc.vector.tensor_scalar_add(
                                    out[dc][:, sl], out[dc][:, sl], b_ap[dc])

            for l in range(nlayers):
                # resident per-layer weights (bufs=1 tags reused across layers)
                w1t = [wlay.tile([128, FF], BF16, tag=f"w1t{i}", name=f"w1t{i}")
                       for i in range(DC)]
                for i in range(DC):
                    r0 = l * D + i * 128
                    nc.sync.dma_start(w1t[i][:], w1pT_d[r0:r0 + 128, :])
                outb = [wlay.tile([128, 1], F32, tag=f"outb{i}", name=f"outb{i}")
                        for i in range(DC)]
                ffb2 = [wlay.tile([128, 1], F32, tag=f"ffb2{i}", name=f"ffb2{i}")
                        for i in range(DC)]
                for i in range(DC):
                    r0 = l * D + i * 128
                    nc.sync.dma_start(outb[i][:], outb_d[r0:r0 + 128, :])
                    nc.sync.dma_start(ffb2[i][:], ffb2_d[r0:r0 + 128, :])
                if not ident_ln:
                    ln1g = [wlay.tile([128, 1], F32, tag=f"ln1g{i}",
                                      name=f"ln1g{i}") for i in range(DC)]
                    ln1ng = [wlay.tile([128, 1], F32, tag=f"ln1ng{i}",
                                       name=f"ln1ng{i}") for i in range(DC)]
                    ln1b = [wlay.tile([128, 1], F32, tag=f"ln1b{i}",
                                      name=f"ln1b{i}") for i in range(DC)]
                    for i in range(DC):
                        r0 = l * D + i * 128
                        nc.sync.dma_start(ln1g[i][:], ln1g_d[r0:r0 + 128, :])
                        nc.sync.dma_start(ln1ng[i][:],
                                          ln1negg_d[r0:r0 + 128, :])
                        nc.sync.dma_start(ln1b[i][:], ln1b_d[r0:r0 + 128, :])
                b1p = wlay.tile([128, FFC], F32, tag="b1p", name="b1p")
                nc.sync.dma_start(b1p[:], b1p_d[l * 128:(l + 1) * 128, :])
                w1f = wlay.tile([8, 128], F32, tag="w1f", name="w1f")
                b1f = wlay.tile([8, 128], F32, tag="b1f", name="b1f")
                w2f = wlay.tile([8, 128], F32, tag="w2f", name="w2f")
                b2r = wlay.tile([8, 1], F32, tag="b2r", name="b2r")
                nc.sync.dma_start(w1f[:], w1f_d[l * 8:(l + 1) * 8, :])
                nc.sync.dma_start(b1f[:], b1f_d[l * 8:(l + 1) * 8, :])
                nc.sync.dma_start(w2f[:], w2f_d[l * 8:(l + 1) * 8, :])
                nc.sync.dma_start(b2r[:], b2r_d[l * 8:(l + 1) * 8, :])
                polwp = [wlay.tile([128, PP], F32, tag=f"polwp{i}", name=f"polwp{i}")
                         for i in range(DC)]
                for i in range(DC):
                    r0 = l * D + i * 128
                    nc.sync.dma_start(polwp[i][:],
                                      polWp_d[r0:r0 + 128, :])
                polb = wlay.tile([PP, 8], F32, tag="polb", name="polb")
                nc.sync.dma_start(polb[:], polb_d[l * PP:(l + 1) * PP, :])

                # ---- xbar = sum_s(x) ; summary = triW_f @ xbar ----
                xbar = [smp.tile([128, 1], F32, tag=f"xbar{i}", name=f"xbar{i}")
                        for i in range(DC)]
                xbw = (S // 128) if l == 0 else NSB
                for dc in range(DC):
                    nc.vector.tensor_reduce(xbar[dc][:],
                                            xbparts[dc][:, 0:xbw],
                                            AX.X, ALU.add)
                trit = [wstr.tile([128, D], F32, tag="wstr", name="wstr")
                        for _ in range(DC)]
                for i in range(DC):
                    r0 = l * D + i * 128
                    nc.sync.dma_start(trit[i][:], triWT_d[r0:r0 + 128, :])
                sum_ps = psb.tile([128, DC], F32, tag="ps", name="sumps")
                for m in range(DC):
                    for kc in range(DC):
                        nc.tensor.matmul(
                            sum_ps[:, m:m + 1],
                            trit[kc][:, m * 128:(m + 1) * 128], xbar[kc][:],
                            start=(kc == 0), stop=(kc == DC - 1))
                summary = smp.tile([128, DC], F32, tag="summary", name="summary")
                with nc.allow_low_precision(reason="f32r storage"):
                    nc.vector.tensor_copy(summary[:], sum_ps[:])

                # ---- pol / dots / impedance / coef chain (tiny) ----
                sm8 = [smp.tile([128, 8], F32, tag=f"sm8{i}", name=f"sm8{i}")
                       for i in range(DC)]
                for dc in range(DC):
                    with nc.allow_low_precision(reason="f32r storage"):
                        nc.vector.tensor_tensor(
                            out=sm8[dc][:],
                            in0=summary[:, dc:dc + 1].to_broadcast([128, 8]),
                            in1=b8[dc][:], op=ALU.mult)
                pol_ps = psb.tile([PP, 8], F32, tag="ps", name="polps")
                for kc in range(DC):
                    nc.tensor.matmul(pol_ps[:], polwp[kc][:], sm8[kc][:],
                                     start=(kc == 0), stop=(kc == DC - 1))
                polpre = smp.tile([PP, 8], F32, tag="polpre", name="polpre")
                nc.vector.tensor_add(polpre[:], pol_ps[:], polb[:])
                pol = smp.tile([PP, 8], F32, tag="pol", name="pol")
                nc.scalar.activation(pol[:], polpre[:], AF.Tanh)
                dots_ps = psb.tile([8, 8], F32, tag="ps", name="dotsps")
                nc.tensor.matmul(dots_ps[:], pol[:], pol[:],
                                 start=True, stop=True)
                dotsU = smp.tile([8, 8], F32, tag="dotsU", name="dotsU")
                nc.vector.tensor_copy(dotsU[:], dots_ps[:])
                dd = smp.tile([8, 8], F32, tag="dd", name="dd")
                nc.vector.tensor_tensor(out=dd[:], in0=dotsU[:], in1=i8[:],
                                        op=ALU.mult)
                diag = smp.tile([8, 1], F32, tag="diag", name="diag")
                nc.vector.tensor_reduce(diag[:], dd[:], AX.X, ALU.add)
                nrm = smp.tile([8, 1], F32, tag="nrm", name="nrm")
                nc.scalar.activation(nrm[:], diag[:], AF.Sqrt)
                nmx = smp.tile([8, 1], F32, tag="nmx", name="nmx")
                nc.vector.tensor_scalar_max(nmx[:], nrm[:], 1e-12)
                ninv = smp.tile([8, 1], F32, tag="ninv", name="ninv")
                nc.vector.reciprocal(ninv[:], nmx[:])
                nr_ps = psb.tile([1, 8], F32, tag="ps", name="nrps")
                nc.tensor.transpose(nr_ps[:], ninv[:], i8[:])
                nr = smp.tile([1, 8], F32, tag="nr", name="nr")
                nc.vector.tensor_copy(nr[:], nr_ps[:])
                nb_ps = psb.tile([8, 8], F32, tag="ps", name="nbps")
                nc.tensor.matmul(nb_ps[:], nr[:], nr[:], start=True, stop=True)
                dotsn = smp.tile([8, 8], F32, tag="dotsn", name="dotsn")
                nc.vector.tensor_tensor(out=dotsn[:], in0=dotsU[:],
                                        in1=nb_ps[:], op=ALU.mult)
                t1 = smp.tile([8, 128], F32, tag="t1", name="t1")
                nc.vector.tensor_tensor(
                    out=t1[:], in0=dotsn[:, :].to_broadcast([8, 8, 16]),
                    in1=w1f[:], op=ALU.mult)
                t2 = smp.tile([8, 128], F32, tag="t2", name="t2")
                nc.vector.tensor_add(t2[:], t1[:], b1f[:])
                hm = smp.tile([8, 128], F32, tag="hm", name="hm")
                if taylor_ok:
                    # gelu(t) ~= t*(0.5 + 0.3989423*t) for tiny |t|
                    g1 = smp.tile([8, 128], F32, tag="g1", name="g1")
                    nc.vector.tensor_scalar(g1[:], t2[:], 0.3989422804014327,
                                            0.5, ALU.mult, ALU.add)
                    nc.vector.tensor_tensor(out=hm[:], in0=g1[:], in1=t2[:],
                                            op=ALU.mult)
                else:
                    nc.scalar.activation(hm[:], t2[:], AF.Gelu)
                t3 = smp.tile([8, 128], F32, tag="t3", name="t3")
                nc.vector.tensor_tensor(out=t3[:], in0=hm[:], in1=w2f[:],
                                        op=ALU.mult)
                impre = smp.tile([8, 8], F32, tag="impre", name="impre")
                nc.vector.tensor_reduce(
                    impre[:], t3[:, :].rearrange("p (j u) -> p j u", u=16),
                    AX.X, ALU.add)
                imp = smp.tile([8, 8], F32, tag="imp", name="imp")
                if taylor_ok:
                    # softplus(v) ~= ln2 + v/2 + v^2/8 for tiny |v|
                    vv = smp.tile([8, 8], F32, tag="vv", name="vv")
                    nc.vector.tensor_scalar(vv[:], impre[:], b2r[:], None,
                                            ALU.add)
                    vsq = smp.tile([8, 8], F32, tag="vsq", name="vsq")
                    nc.vector.tensor_tensor(out=vsq[:], in0=vv[:], in1=vv[:],
                                            op=ALU.mult)
                    w8 = smp.tile([8, 8], F32, tag="w8", name="w8")
                    nc.vector.tensor_scalar(w8[:], vsq[:], 0.125, LN2_2,
                                            ALU.mult, ALU.add)
                    nc.vector.scalar_tensor_tensor(
                        out=imp[:], in0=vv[:], scalar=0.5, in1=w8[:],
                        op0=ALU.mult, op1=ALU.add)
                else:
                    ex = smp.tile([8, 8], F32, tag="ex", name="ex")
                    nc.scalar.activation(ex[:], impre[:], AF.Exp, bias=b2r[:])
                    exp1 = smp.tile([8, 8], F32, tag="exp1", name="exp1")
                    nc.vector.tensor_scalar_add(exp1[:], ex[:], 1.0)
                    nc.scalar.activation(imp[:], exp1[:], AF.Ln)
                ip1 = smp.tile([8, 8], F32, tag="ip1", name="ip1")
                nc.vector.tensor_scalar_add(ip1[:], imp[:], 1.0)
                rcoef = smp.tile([8, 8], F32, tag="rcoef", name="rcoef")
                nc.vector.reciprocal(rcoef[:], ip1[:])
                coefm = smp.tile([8, 8], F32, tag="coefm", name="coefm")
                nc.vector.tensor_tensor(out=coefm[:], in0=rcoef[:],
                                        in1=mask01[:], op=ALU.mult)
                cp = smp.tile([8, 8], F32R, tag="cp", name="cp")
                with nc.allow_low_precision(reason="f32r storage"):
                    nc.vector.tensor_add(cp[:], coefm[:], i8[:])
                if debug_outs and l == 0:
                    nc.sync.dma_start(dbg["coef"][:, :], coefm[:].bitcast(F32))
                cpe = smp.tile([8, D], F32R, tag="cpe", name="cpe")
                with nc.allow_low_precision(reason="f32r storage"):
                    nc.vector.tensor_copy(cpe[:],
                                          cp[:, :].to_broadcast([8, 8, KH]))

                # ---- Mmix = kron(I + coef, I64); W2p = Mmix^T out_W^T;
                #      W3 = triW_f^T W2p ----
                Mmix = [wlay.tile([128, D], F32R, tag=f"Mmix{i}", name=f"Mmix{i}")
                        for i in range(DC)]
                for ic in range(DC):
                    mps = psb.tile([128, SB], F32, tag="ps", name="ps")
                    nc.tensor.matmul(mps[:], e8r[:, ic * 128:(ic + 1) * 128],
                                     cpe[:], start=True, stop=True)
                    with nc.allow_low_precision(reason="f32r storage"):
                        nc.vector.tensor_tensor(out=Mmix[ic][:], in0=mps[:],
                                                in1=kron[ic][:], op=ALU.mult)
                outwt = [wstr.tile([128, D], F32R, tag="wstr", name="wstr")
                         for _ in range(DC)]
                for i in range(DC):
                    r0 = l * D + i * 128
                    nc.sync.dma_start(outwt[i][:],
                                      outWT_d[r0:r0 + 128, :].bitcast(F32R))
                W2p = [wlay.tile([128, D], F32R, tag=f"W2p{i}", name=f"W2p{i}")
                       for i in range(DC)]
                for m in range(DC):
                    wps = psb.tile([128, SB], F32, tag="ps", name="ps")
                    for kc in range(DC):
                        nc.tensor.matmul(
                            wps[:], Mmix[kc][:, m * 128:(m + 1) * 128],
                            outwt[kc][:], start=(kc == 0), stop=(kc == DC - 1))
                    with nc.allow_low_precision(reason="f32r storage"):
                        nc.vector.tensor_copy(W2p[m][:], wps[:])
                triN = [wstr.tile([128, D], F32R, tag="wstr", name="wstr")
                        for _ in range(DC)]
                for i in range(DC):
                    r0 = l * D + i * 128
                    nc.sync.dma_start(triN[i][:],
                                      triWN_d[r0:r0 + 128, :].bitcast(F32R))
                W3 = [wlay.tile([128, D], F32R, tag=f"Mmix{i}", name=f"W3{i}")
                      for i in range(DC)]
                for m in range(DC):
                    wps = psb.tile([128, SB], F32, tag="ps", name="ps")
                    for kc in range(DC):
                        nc.tensor.matmul(
                            wps[:], triN[kc][:, m * 128:(m + 1) * 128],
                            W2p[kc][:], start=(kc == 0), stop=(kc == DC - 1))
                    with nc.allow_low_precision(reason="f32r storage"):
                        nc.vector.tensor_copy(W3[m][:], wps[:])

                # ---- out-proj (all blocks) -> LN1 (all blocks) ----
                for sblk in range(NSB):
                    sl = slice(sblk * SB, (sblk + 1) * SB)
                    # out projection + residual + bias -> Bt (pre-LN)
                    for m in range(DC):
                        ps = psb.tile([128, SB], F32, tag="ps", name="ps")
                        for kc in range(DC):
                            nc.tensor.matmul(
                                ps[:], W3[kc][:, m * 128:(m + 1) * 128],
                                A[kc][:, sl], start=(kc == 0),
                                stop=(kc == DC - 1))
                        with nc.allow_low_precision(reason="f32r storage"):
                            nc.vector.scalar_tensor_tensor(
                                out=Bt[m][:, sl], in0=ps[:],
                                scalar=outb[m][:], in1=A[m][:, sl],
                                op0=ALU.add, op1=ALU.add)
                    if ident_ln:
                        emit_ln_fast(Bt, Ct, sblk)

                # ---- per s-block: FF + SuanLi ----
                for sblk in range(NSB):
                    sl = slice(sblk * SB, (sblk + 1) * SB)
                    if ident_ln:
                        # FF: Bt <- Ct + W2 @ gelu(W1 @ Ct + b1) + b2
                        pso = [psff.tile([128, SB], F32, tag=f"ffo{i}",
                                         name=f"ffo{i}")
                               for i in range(DC)]
                        for ffc in range(FFC):
                            ps1 = psb.tile([128, SB], F32, tag="ps", name="ps")
                            for kc in range(DC):
                                nc.tensor.matmul(
                                    ps1[:],
                                    w1t[kc][:, ffc * 128:(ffc + 1) * 128],
                                    Ct[kc][:, sl], start=(kc == 0),
                                    stop=(kc == DC - 1))
                            gt = gep.tile([128, SB], BF16, tag="gelu",
                                          name="gelu")
                            nc.scalar.activation(gt[:], ps1[:], AF.Gelu,
                                                 bias=b1p[:, ffc:ffc + 1])
                            w2s = wstr.tile([128, D], BF16, tag="wstr",
                                            name="wstr")
                            nc.sync.dma_start(
                                w2s[:],
                                ffW2T_d[l * FF + ffc * 128:
                                        l * FF + (ffc + 1) * 128, :])
                            for dc in range(DC):
                                nc.tensor.matmul(
                                    pso[dc][:],
                                    w2s[:, dc * 128:(dc + 1) * 128],
                                    gt[:], start=(ffc == 0),
                                    stop=(ffc == FFC - 1))
                        for dc in range(DC):
                            with nc.allow_low_precision(reason="f32r"):
                                nc.vector.scalar_tensor_tensor(
                                    out=Bt[dc][:, sl], in0=pso[dc][:],
                                    scalar=ffb2[dc][:], in1=Ct[dc][:, sl],
                                    op0=ALU.add, op1=ALU.add)

                        # SuanLi: A <- Bt + 0.25*(1+tanh(dis))*(1+tanh(chg))*A
                        cgp = []
                        for m in range(DC):
                            psc = psb.tile([128, SB], F32, tag="ps", name="ps")
                            for kc in range(DC):
                                nc.tensor.matmul(
                                    psc[:],
                                    chgwt[kc][:, m * 128:(m + 1) * 128],
                                    A[kc][:, sl], start=(kc == 0),
                                    stop=(kc == DC - 1))
                            tcg = evp.tile([128, SB], F32, tag="sig",
                                           name="sig")
                            nc.scalar.activation(tcg[:], psc[:], AF.Tanh,
                                                 bias=chgbh[m][:], scale=0.5)
                            cgt = cgpp.tile([128, SB], BF16, tag="cgp",
                                            name="cgp")
                            with nc.allow_low_precision(reason="bf16"):
                                nc.vector.scalar_tensor_tensor(
                                    out=cgt[:], in0=tcg[:], scalar=1.0,
                                    in1=A[m][:, sl], op0=ALU.add,
                                    op1=ALU.mult)
                            cgp.append(cgt)
                        for m in range(DC):
                            psd = psb.tile([128, SB], F32, tag="ps", name="ps")
                            for kc in range(DC):
                                nc.tensor.matmul(
                                    psd[:],
                                    diswt[kc][:, m * 128:(m + 1) * 128],
                                    Bt[kc][:, sl], start=(kc == 0),
                                    stop=(kc == DC - 1))
                            tsd = evp.tile([128, SB], F32, tag="sig",
                                           name="sig")
                            nc.scalar.activation(tsd[:], psd[:], AF.Tanh,
                                                 bias=disbh[m][:], scale=0.5)
                            u1 = evp.tile([128, SB], F32, tag="u1", name="u1")
                            nc.vector.scalar_tensor_tensor(
                                out=u1[:], in0=tsd[:], scalar=1.0,
                                in1=cgp[m][:], op0=ALU.add, op1=ALU.mult)
                            with nc.allow_low_precision(reason="f32r storage"):
                                nc.vector.scalar_tensor_tensor(
                                    out=A[m][:, sl], in0=u1[:], scalar=0.25,
                                    in1=Bt[m][:, sl], op0=ALU.mult,
                                    op1=ALU.add,
                                    accum_out=xbparts[m][:, sblk:sblk + 1])

                if not ident_ln:
                    # fallback: full LN1 (affine) + LN2 + unfused FF/SuanLi
                    emit_ln(Bt, Bt, [t[:] for t in ln1g],
                            [t[:] for t in ln1ng], [t[:] for t in ln1b])
                    emit_ln(Bt, Ct, [onecol[:]] * DC, [negcol[:]] * DC, None)
                    for sblk in range(NSB):
                        sl = slice(sblk * SB, (sblk + 1) * SB)
                        pso = [psff.tile([128, SB], F32, tag=f"ffo{i}",
                                         name=f"ffo{i}")
                               for i in range(DC)]
                        for ffc in range(FFC):
                            ps1 = psb.tile([128, SB], F32, tag="ps", name="ps")
                            for kc in range(DC):
                                nc.tensor.matmul(
                                    ps1[:],
                                    w1t[kc][:, ffc * 128:(ffc + 1) * 128],
                                    Ct[kc][:, sl], start=(kc == 0),
                                    stop=(kc == DC - 1))
                            gt = gep.tile([128, SB], BF16, tag="gelu",
                                          name="gelu")
                            nc.scalar.activation(gt[:], ps1[:], AF.Gelu,
                                                 bias=b1p[:, ffc:ffc + 1])
                            w2s = wstr.tile([128, D], BF16, tag="wstr",
                                            name="wstr")
                            nc.sync.dma_start(
                                w2s[:],
                                ffW2T_d[l * FF + ffc * 128:
                                        l * FF + (ffc + 1) * 128, :])
                            for dc in range(DC):
                                nc.tensor.matmul(
                                    pso[dc][:],
                                    w2s[:, dc * 128:(dc + 1) * 128],
                                    gt[:], start=(ffc == 0),
                                    stop=(ffc == FFC - 1))
                        for dc in range(DC):
                            with nc.allow_low_precision(reason="f32r"):
                                nc.vector.scalar_tensor_tensor(
                                    out=Bt[dc][:, sl], in0=pso[dc][:],
                                    scalar=ffb2[dc][:], in1=Bt[dc][:, sl],
                                    op0=ALU.add, op1=ALU.add)
                        cgp = []
                        for m in range(DC):
                            psc = psb.tile([128, SB], F32, tag="ps", name="ps")
                            for kc in range(DC):
                                nc.tensor.matmul(
                                    psc[:],
                                    chgwt[kc][:, m * 128:(m + 1) * 128],
                                    A[kc][:, sl], start=(kc == 0),
                                    stop=(kc == DC - 1))
                            tcg = evp.tile([128, SB], F32, tag="sig",
                                           name="sig")
                            nc.scalar.activation(tcg[:], psc[:], AF.Tanh,
                                                 bias=chgbh[m][:], scale=0.5)
                            cgt = cgpp.tile([128, SB], BF16, tag="cgp",
                                            name="cgp")
                            with nc.allow_low_precision(reason="bf16"):
                                nc.vector.scalar_tensor_tensor(
                                    out=cgt[:], in0=tcg[:], scalar=1.0,
                                    in1=A[m][:, sl], op0=ALU.add,
                                    op1=ALU.mult)
                            cgp.append(cgt)
                        for m in range(DC):
                            psd = psb.tile([128, SB], F32, tag="ps", name="ps")
                            for kc in range(DC):
                                nc.tensor.matmul(
                                    psd[:],
                                    diswt[kc][:, m * 128:(m + 1) * 128],
                                    Bt[kc][:, sl], start=(kc == 0),
                                    stop=(kc == DC - 1))
                            tsd = evp.tile([128, SB], F32, tag="sig",
                                           name="sig")
                            nc.scalar.activation(tsd[:], psd[:], AF.Tanh,
                                                 bias=disbh[m][:], scale=0.5)
                            u1 = evp.tile([128, SB], F32, tag="u1", name="u1")
                            nc.vector.scalar_tensor_tensor(
                                out=u1[:], in0=tsd[:], scalar=1.0,
                                in1=cgp[m][:], op0=ALU.add, op1=ALU.mult)
                            with nc.allow_low_precision(reason="f32r storage"):
                                nc.vector.scalar_tensor_tensor(
                                    out=A[m][:, sl], in0=u1[:], scalar=0.25,
                                    in1=Bt[m][:, sl], op0=ALU.mult,
                                    op1=ALU.add,
                                    accum_out=xbparts[m][:, sblk:sblk + 1])

                if debug_outs and l == 0:
                    for dc in range(DC):
                        nc.sync.dma_start(
                            dbg["x1"][dc * 128:(dc + 1) * 128, :],
                            Ct[dc][:])
                if debug_outs and l in (0, L - 1):
                    nm = "x3" if l == 0 else "x4"
                    for dc in range(DC):
                        nc.sync.dma_start(
                            dbg[nm][dc * 128:(dc + 1) * 128, :],
                            A[dc][:].bitcast(F32))

        if not do_scan:
            pooled0 = smp.tile([128, 1], F32, tag="pool0", name="pool0")
            for dc in range(DC):
                nc.vector.tensor_reduce(pooled0[:], xbparts[dc][:, :],
                                        AX.X, ALU.add)
                nc.sync.dma_start(pooled_d[dc * 128:(dc + 1) * 128, :],
                                  pooled0[:])
        # ---------- ZuoEr scan (tanh-folded sigmoids) ----------
        if do_scan:
         with tc.tile_pool(name="scanp", bufs=1) as scp, \
             tc.tile_pool(name="psscan", bufs=2, space="PSUM") as psscan:
            zwr = [scp.tile([128, 65], F32R, tag=f"zwr{i}", name=f"zwr{i}")
                   for i in range(DC)]
            for i in range(DC):
                nc.sync.dma_start(
                    zwr[i][:], zwr33_d[i * 128:(i + 1) * 128, :].bitcast(F32R))
            zwrbh = scp.tile([65, 1], F32, tag="zwrb", name="zwrb")
            nc.sync.dma_start(zwrbh[:], zwrbh_d[:, :])
            zpwt = scp.tile([MEM, D], F32R, tag="zpwt", name="zpwt")
            nc.sync.dma_start(zpwt[:], zpWTh_d[:, :].bitcast(F32R))
            zpb = [scp.tile([128, 1], F32, tag=f"zpb{i}", name=f"zpb{i}") for i in range(DC)]
            zob = [scp.tile([128, 1], F32, tag=f"zob{i}", name=f"zob{i}") for i in range(DC)]
            for i in range(DC):
                nc.sync.dma_start(zpb[i][:], zpb_d[i * 128:(i + 1) * 128, :])
                nc.sync.dma_start(zob[i][:], zob_d[i * 128:(i + 1) * 128, :])

            # tw = tanh((ww_pre)/2), tr = tanh((rw_pre)/2), mval' = mval/2
            tw = scp.tile([MEM, S], F32, tag="scrow", name="tw", bufs=4)
            tr = scp.tile([MEM, S], F32, tag="scrow", name="tr", bufs=4)
            mval = scp.tile([1, S], F32R, tag="mval", name="mval")
            for sblk in range(NSB):
                sl = slice(sblk * SB, (sblk + 1) * SB)
                ps = psscan.tile([65, SB], F32, tag="sps", name="zwrps")
                for kc in range(DC):
                    nc.tensor.matmul(ps[:], zwr[kc][:], A[kc][:, sl],
                                     start=(kc == 0), stop=(kc == DC - 1))
                nc.scalar.activation(tw[:, sl], ps[0:MEM, :], AF.Tanh,
                                     bias=zwrbh[0:MEM, :], scale=0.5)
                nc.scalar.activation(tr[:, sl], ps[32:32 + MEM, :],
                                     AF.Tanh, bias=zwrbh[32:32 + MEM, :],
                                     scale=0.5)
                with nc.allow_low_precision(reason="f32r storage"):
                    nc.scalar.activation(mval[:, sl], ps[64:65, :],
                                         AF.Identity, bias=zwrbh[64:65, :])
            # ascan = 1 - ww = 0.5 - 0.5*tw ; bscan = ww*mval = (1+tw)*mval'
            ascan = scp.tile([MEM, S], F32, tag="scrow", name="ascan", bufs=4)
            nc.vector.tensor_scalar(ascan[:], tw[:], -0.5, 0.5,
                                    ALU.mult, ALU.add)
            bscan = scp.tile([MEM, S], F32, tag="scrow", name="bscan", bufs=4)
            for sblk in range(NSB):
                sl = slice(sblk * SB, (sblk + 1) * SB)
                mb = psscan.tile([MEM, SB], F32, tag="sps", name="mbps")
                nc.tensor.matmul(mb[:], ones116[:], mval[:, sl],
                                 start=True, stop=True)
                nc.vector.scalar_tensor_tensor(
                    out=bscan[:, sl], in0=tw[:, sl], scalar=1.0, in1=mb[:],
                    op0=ALU.add, op1=ALU.mult)
            As = scp.tile([MEM, S], F32, tag="scrow", name="As", bufs=4)
            nc.vector.tensor_tensor_scan(As[:], ascan[:], bscan[:], 0.0,
                                         ALU.mult, ALU.add)
            Ex = scp.tile([MEM, S], F32, tag="scrow", name="Ex", bufs=4)
            nc.vector.memset(Ex[:, 0:1], 0.0)
            nc.vector.tensor_copy(Ex[:, 1:S], As[:, 0:S - 1])
            if debug_outs:
                nc.sync.dma_start(dbg["scan"][:, :], Ex[:])
            # rw * M = 0.5*(1+tr)*Ex ; the 0.5 is folded into zpWTh
            rwE = scp.tile([MEM, S], F32R, tag="rwE", name="rwE")
            with nc.allow_low_precision(reason="f32r storage"):
                nc.vector.scalar_tensor_tensor(
                    out=rwE[:], in0=tr[:], scalar=1.0, in1=Ex[:],
                    op0=ALU.add, op1=ALU.mult)
            # mem_vec^T -> Bt (f32r; Bt is dead after the layer loop)
            for dc in range(DC):
                for sblk in range(NSB):
                    sl = slice(sblk * SB, (sblk + 1) * SB)
                    ps = psb.tile([128, SB], F32, tag="ps", name="ps")
                    nc.tensor.matmul(ps[:], zpwt[:, dc * 128:(dc + 1) * 128],
                                     rwE[:, sl], start=True, stop=True)
                    nc.scalar.activation(Bt[dc][:, sl], ps[:],
                                         AF.Identity, bias=zpb[dc][:])
            # fused = tanh(zo_W @ [x; mem] + zo_b) ; accumulate pooled sums.
            zowt = [scp.tile([128, D], F32R, tag=f"zowt{i}", name=f"zowt{i}")
                    for i in range(2 * DC)]
            for kc in range(2 * DC):
                nc.sync.dma_start(
                    zowt[kc][:], zoWT_d[kc * 128:(kc + 1) * 128, :]
                    .bitcast(F32R))
            poolparts = [scp.tile([128, NSB], F32, tag=f"poolp{i}", name=f"poolp{i}")
                         for i in range(DC)]
            for dc in range(DC):
                for sblk in range(NSB):
                    sl = slice(sblk * SB, (sblk + 1) * SB)
                    ps = psb.tile([128, SB], F32, tag="ps", name="ps")
                    for kc in range(2 * DC):
                        rhs = (A[kc][:, sl] if kc < DC
                               else Bt[kc - DC][:, sl])
                        nc.tensor.matmul(
                            ps[:], zowt[kc][:, dc * 128:(dc + 1) * 128],
                            rhs, start=(kc == 0), stop=(kc == 2 * DC - 1))
                    nc.scalar.activation(
                        Ct[dc][:, sl], ps[:], AF.Tanh, bias=zob[dc][:],
                        accum_out=poolparts[dc][:, sblk:sblk + 1])
            if debug_outs:
                for dc in range(DC):
                    nc.gpsimd.dma_start(dbg["ys"][dc * 128:(dc + 1) * 128, :],
                                        Ct[dc][:])
            pooled = [scp.tile([128, 1], F32, tag=f"pool{i}", name=f"pool{i}")
                      for i in range(DC)]
            for dc in range(DC):
                nc.vector.tensor_reduce(pooled[dc][:], poolparts[dc][:],
                                        AX.X, ALU.add)
                nc.sync.dma_start(pooled_d[dc * 128:(dc + 1) * 128, :],
                                  pooled[dc][:])

    nc.compile()
    return nc


def _host_prep(inputs):
    """Build the shared (weight) arrays in device layout + per-core idx."""
    import ml_dtypes
    f = lambda x: np.ascontiguousarray(np.asarray(x, dtype=np.float32))
    bf = lambda x: np.ascontiguousarray(
        np.asarray(x, np.float32).astype(ml_dtypes.bfloat16))
    ids = np.asarray(inputs["input_ids"]).astype(np.int64)

    shared = {}
    shared["emb"] = f(inputs["emb"])
    shared["posN"] = f(np.asarray(inputs["pos_emb"])[:S])
    shared["i128"] = np.eye(128, dtype=np.float32)

    triW = f(inputs["tri_W"])           # [L, 8, K, D]
    cosf = np.cos(f(inputs["res_freq"]) * np.pi)  # [L, 8, K]
    triWf = triW * cosf[:, :, :, None]
    shared["triWT"] = f(np.concatenate(
        [triWf[l].reshape(D, D).T for l in range(L)], axis=0))
    shared["triWN"] = f(np.concatenate(
        [triWf[l].reshape(D, D) for l in range(L)], axis=0))

    outW = f(inputs["out_W"])           # [L, D, D]
    shared["outWT"] = f(np.concatenate([outW[l].T for l in range(L)], axis=0))
    shared["outb"] = f(inputs["out_b"]).reshape(L * D, 1)

    g1 = f(inputs["ln1_g"]).reshape(L * D)
    b1 = f(inputs["ln1_b"]).reshape(L * D)
    shared["ln1g"] = g1.reshape(L * D, 1)
    shared["ln1negg"] = (-g1).reshape(L * D, 1)
    shared["ln1b"] = b1.reshape(L * D, 1)
    ident_ln = bool(np.all(g1 == 1.0) and np.all(b1 == 0.0))

    w1i = f(inputs["imp_w1"])           # [L, 16]
    b1i = f(inputs["imp_b1"])
    w2i = f(inputs["imp_w2"])
    b2i = f(inputs["imp_b2"])           # [L]
    # Taylor forms are accurate when the pol-chain pre-activations are tiny
    # (|dots| <= 1 by construction, so bounded by the weight magnitudes).
    tmax = 16 * np.abs(w1i).max() / 16 + np.abs(b1i).max()
    vmax = 16 * 0.5 * (tmax + 0.4 * tmax ** 2) * np.abs(w2i).max() \
        + np.abs(b2i).max()
    taylor_ok = bool(tmax < 0.25 and vmax < 0.5)

    ffg = f(inputs["ff_ln_g"])          # [L, D]
    ffb = f(inputs["ff_ln_b"])          # [L, D]
    W1 = f(inputs["ff_W1"])             # [L, FF, D]
    b1_ff = f(inputs["ff_b1"])          # [L, FF]
    W1p = W1 * ffg[:, None, :]
    b1p = b1_ff + np.einsum("lfd,ld->lf", W1, ffb)
    shared["w1pT"] = bf(np.concatenate([W1p[l].T for l in range(L)], axis=0))
    shared["b1p"] = f(np.concatenate(
        [b1p[l].reshape(FFC, 128).T for l in range(L)], axis=0))
    W2 = f(inputs["ff_W2"])             # [L, D, FF]
    shared["ffW2T"] = bf(np.concatenate([W2[l].T for l in range(L)], axis=0))
    shared["ffb2"] = f(inputs["ff_b2"]).reshape(L * D, 1)

    shared["disWT"] = f(inputs["dis_W"]).T
    shared["chgWT"] = f(inputs["chg_W"]).T
    shared["disbh"] = (0.5 * f(inputs["dis_b"])).reshape(D, 1)
    shared["chgbh"] = (0.5 * f(inputs["chg_b"])).reshape(D, 1)

    ztW = f(inputs["zt_W"])             # [D, D]
    wbar = ztW.mean(0)                  # [D]
    bbar = float(f(inputs["zt_b"]).mean())
    zwr65 = np.zeros((D, 65), np.float32)
    zwr65[:, 0:16] = f(inputs["zw_W"]).T
    zwr65[:, 32:48] = f(inputs["zr_W"]).T
    zwr65[:, 64] = 0.5 * wbar           # mval/2 (ww's 0.5 folded here)
    shared["zwr33"] = f(zwr65)          # [D, 65]
    zwrbh = np.zeros((65, 1), np.float32)
    zwrbh[0:16, 0] = 0.5 * np.asarray(inputs["zw_b"], np.float32)
    zwrbh[32:48, 0] = 0.5 * np.asarray(inputs["zr_b"], np.float32)
    zwrbh[64, 0] = 0.5 * bbar
    shared["zwrbh"] = zwrbh
    shared["zpWTh"] = f(0.5 * np.asarray(inputs["zp_W"]).T)   # rw's 0.5
    shared["zpb"] = f(inputs["zp_b"]).reshape(D, 1)
    shared["zoWT"] = f(np.asarray(inputs["zo_W"]).T)      # [2D, D]
    shared["zob"] = f(inputs["zo_b"]).reshape(D, 1)

    shared["i8"] = np.eye(8, dtype=np.float32)
    e8 = np.zeros((8, D), np.float32)
    for i in range(8):
        e8[i, i * KH:(i + 1) * KH] = 1.0
    shared["e8"] = e8
    shared["kron"] = np.kron(np.ones((8, 8), np.float32),
                             np.eye(KH, dtype=np.float32))
    shared["mask01"] = (0.1 * (1.0 - np.eye(8))).astype(np.float32)

    shared["w1full"] = f(np.concatenate(
        [np.tile(w1i[l], (8, 8)) for l in range(L)], axis=0))  # [L*8, 128]
    shared["b1full"] = f(np.concatenate(
        [np.tile(b1i[l], (8, 8)) for l in range(L)], axis=0))
    shared["w2full"] = f(np.concatenate(
        [np.tile(w2i[l], (8, 8)) for l in range(L)], axis=0))
    shared["b2rep"] = f(np.repeat(b2i, 8)).reshape(L * 8, 1)

    polW = f(inputs["pol_W"])           # [L, 8, P, K]
    polWp = np.zeros((L * D, PP), np.float32)
    for l in range(L):
        for h in range(8):
            polWp[l * D + h * KH:(l * D) + (h + 1) * KH, :] = \
                polW[l, h].T / float(S)
    shared["polWp"] = f(polWp)
    polb = f(inputs["pol_b"])           # [L, 8, P]
    shared["polb"] = f(np.concatenate(
        [polb[l].T for l in range(L)], axis=0))   # [L*P, 8]

    shared["onesv"] = np.ones((128, 128), np.float32)
    shared["onesb"] = np.ones((128, 1), ml_dtypes.bfloat16)

    b8 = np.zeros((D, 8), np.float32)
    for h in range(8):
        b8[h * KH:(h + 1) * KH, h] = 1.0
    shared["b8"] = b8

    idx_per_core = []
    for c in range(NCORES):
        idx = ids[c].reshape(S // 128, 128).T.astype(np.int32)  # [128, S/128]
        idx_per_core.append(np.ascontiguousarray(idx))
    return shared, idx_per_core, (ident_ln, taylor_ok)


_CACHE = {}


def get_nc(fast_flags, debug_outs=False, nlayers=L, do_scan=True):
    key = (fast_flags, debug_outs, nlayers, do_scan)
    if key not in _CACHE:
        _CACHE[key] = _build(fast_flags, debug_outs, nlayers, do_scan)
    return _CACHE[key]


def run_device(inputs, debug_outs=False, trace=False):
    shared, idx_per_core, fast_flags = _host_prep(inputs)
    nc = get_nc(fast_flags, debug_outs)
    in_maps = [dict(shared, idx=idx_per_core[c]) for c in range(NCORES)]
    res = bass_utils.run_bass_kernel_spmd(
        nc, in_maps, core_ids=list(range(NCORES)), trace=trace)
    return res


def _host_finish(inputs, pooled):
    """pooled: [B, D] sums over s (not yet divided). Returns [B, C] f32."""
    x = pooled.astype(np.float64) / float(S)
    g = np.asarray(inputs["cls_ln_g"], np.float64)
    b = np.asarray(inputs["cls_ln_b"], np.float64)
    m = x.mean(-1, keepdims=True)
    v = ((x - m) ** 2).mean(-1, keepdims=True)
    xn = (x - m) / np.sqrt(v + 1e-5) * g + b
    W = np.asarray(inputs["cls_W"], np.float64)
    bb = np.asarray(inputs["cls_b"], np.float64)
    return (xn @ W.T + bb).astype(np.float32)


def kernel(**inputs) -> np.ndarray:
    res = run_device(inputs, debug_outs=False, trace=False)
    pooled = np.stack([res.results[c]["pooled"][:, 0] for c in range(NCORES)])
    return _host_finish(inputs, pooled)
